# revision 12
# baseline (speedup 1.0000x reference)
"""Grouped-GEMM (MoE expert FFN) kernel for 8 Trainium2 NeuronCores.

Problem: out[e, m, n] = sum_k x[e, m, k] * w[e, n, k] for m < m_sizes[e],
         zero elsewhere.  E=8, MAX_M=2048, K=2048, N=8192, fp32.

Strategy
--------
* N-split sharding: every core computes ALL experts' valid row-tiles
  against its own (N/8)-wide column slice of each expert's weights.
  Per-core work is identical by construction (perfect load balance) and
  per-core weight traffic hits the fp16 floor (each weight element is
  read exactly once fleet-wide).
* Only ceil(m_e/128) row-tiles per expert are computed (the program is
  specialized to the runtime m_sizes tuple and cached per tuple).
* fp16 operands: full-clock fp16 matmul cadence measured at ~215 ns per
  512-moving instruction (vs 227 ns fp32r) and half the HBM traffic.
* 2 of 16 k-tiles (256 of 2048 K columns) run as fp8e4 DoubleRow
  matmuls (2x PE rate).  Empirical max rel err 0.0143 vs the 2e-2 gate.
  fp8 operands are pre-scaled (x*16, w*256) to stay in e4m3's normal
  range; the 1/4096 descale folds into the PSUM-merge eviction op.
* Host pre-transposes AND pre-swizzles x/w so every device DMA is one
  large contiguous line per partition.
* PSUM: per row-tile 2 fp16-accum banks + 2 fp8-accum banks; a single
  DVE scalar_tensor_tensor per output half merges them (ps8/4096 + ps16)
  straight into the SBUF out tile; outputs stream back per row-tile.
"""
import math
import os
import sys
import types

import numpy as np
import ml_dtypes

import concourse.bass as bass
import concourse.tile as tile
from concourse import bacc, mybir
from concourse.alu_op_type import AluOpType
from concourse.bass_utils import run_bass_kernel_spmd

P = 128          # partition dim / k-tile
N_CORES = 8
MM_N = 512       # moving free dim per matmul (one PSUM bank of fp32)
N8 = 2           # leading k-tiles computed in fp8e4 DoubleRow (1 DR matmul)
SX = 16.0        # fp8 pre-scale for x
SW = 256.0       # fp8 pre-scale for w
DESCALE = 1.0 / (SX * SW)

LAST_RESULT = None   # BassKernelResults of the most recent run (for tests)


def _install_profile_shim():
    """The agent image's antenv stub lacks axon_hooks; provide it so
    BASS_TRACE=1 profiling works instead of crashing."""
    if "antenv.axon_hooks" in sys.modules:
        return
    try:
        from trn_agent_boot.trn_boot import _ntff_profile_via_ctypes
        hook = _ntff_profile_via_ctypes("/opt/axon/libaxon_pjrt.so")
        mod = types.ModuleType("antenv.axon_hooks")
        mod.get_axon_ntff_profile_hook = lambda: hook
        sys.modules["antenv.axon_hooks"] = mod
        import antenv
        antenv.axon_hooks = mod
    except Exception:
        pass


def to_fp16(a: np.ndarray) -> np.ndarray:
    return np.ascontiguousarray(a).astype(np.float16)


def to_fp8(a: np.ndarray, scale: float) -> np.ndarray:
    return np.ascontiguousarray(
        (np.ascontiguousarray(a) * scale).astype(ml_dtypes.float8_e4m3))


def build_nc(t_list, K, NC_N, psum_bufs=8, w_bufs=32, w8_bufs=3, x_bufs=6,
             x8_bufs=6, out_bufs=4, warmups=12):
    """Build the SPMD program for per-segment row-tile counts t_list."""
    KK = K // P          # total k-tiles
    KK16 = KK - N8       # fp16 k-tiles
    NSEG = len(t_list)
    NH = NC_N // MM_N
    n_chunks = sum(t_list)     # one x chunk per row-tile
    R = P * n_chunks

    nc = bacc.Bacc("TRN2", target_bir_lowering=False, debug=False,
                   num_devices=N_CORES)
    # per chunk: row (chunk*P + p) = partition p's line [kk16, m]
    xsw16 = nc.dram_tensor("xsw16", [n_chunks * P, KK16 * P],
                           mybir.dt.float16, kind="ExternalInput").ap()
    # per chunk: row (chunk*P + p) = partition p's line [pair, m]
    xsw8 = nc.dram_tensor("xsw8", [n_chunks * P, N8, P],
                          mybir.dt.float8e4, kind="ExternalInput").ap()
    # per (segment, k-tile16): row ((seg*KK16 + kk)*P + p)
    wsw16 = nc.dram_tensor("wsw16", [NSEG * KK16 * P, NC_N],
                           mybir.dt.float16, kind="ExternalInput").ap()
    # per segment: row (seg*P + p) = partition p's line [pair, n]
    wsw8 = nc.dram_tensor("wsw8", [NSEG * P, N8, NC_N],
                          mybir.dt.float8e4, kind="ExternalInput").ap()
    out = nc.dram_tensor("out", [R, NC_N], mybir.dt.float32,
                         kind="ExternalOutput").ap()

    with tile.TileContext(nc) as tc:
        with tc.tile_pool(name="wp", bufs=w_bufs) as wp, \
             tc.tile_pool(name="w8p", bufs=w8_bufs) as w8p, \
             tc.tile_pool(name="xp", bufs=x_bufs) as xp, \
             tc.tile_pool(name="x8p", bufs=x8_bufs) as x8p, \
             tc.tile_pool(name="op", bufs=out_bufs) as op, \
             tc.tile_pool(name="pp", bufs=psum_bufs, space="PSUM") as pp, \
             tc.tile_pool(name="wu", bufs=1) as wu:
            # PE warmup: short dummy matmuls keep the HAM activity monitor
            # engaged so the PE clock ramps while the first DMAs land.
            # Rotating psum tiles avoids WAW chaining.
            wa_r = wu.tile([P, P], mybir.dt.float16, tag="war")
            nc.vector.memset(wa_r[:], 0.0)
            wpss = [pp.tile([P, MM_N], mybir.dt.float32, tag="ps",
                            name="wps") for _ in range(4)]
            for i in range(warmups):
                nc.tensor.matmul(wpss[i % 4][:, :P], wa_r[:], wa_r[:],
                                 start=True, stop=True)
            row0 = 0
            chunk = 0
            for seg, T in enumerate(t_list):
                w8_t = w8p.tile([P, N8, NC_N], mybir.dt.float8e4, tag="w8")
                nc.sync.dma_start(
                    out=w8_t[:], in_=wsw8[seg * P:(seg + 1) * P, :, :])
                w_ts = []
                for kk in range(KK16):
                    w_t = wp.tile([P, NC_N], mybir.dt.float16, tag="w")
                    eng = nc.gpsimd if (seg == 0 and kk % 2) else nc.sync
                    eng.dma_start(
                        out=w_t[:],
                        in_=wsw16[(seg * KK16 + kk) * P:
                                  (seg * KK16 + kk + 1) * P, :])
                    w_ts.append(w_t)
                for c0 in range(T):
                    m0 = row0 + c0 * P
                    x8_t = x8p.tile([P, N8, P], mybir.dt.float8e4, tag="x8")
                    nc.scalar.dma_start(
                        out=x8_t[:], in_=xsw8[chunk * P:(chunk + 1) * P, :, :])
                    x_t = xp.tile([P, KK16 * P], mybir.dt.float16, tag="x")
                    nc.scalar.dma_start(
                        out=x_t[:], in_=xsw16[chunk * P:(chunk + 1) * P, :])
                    chunk += 1
                    o_t = op.tile([P, NC_N], mybir.dt.float32, tag="o")
                    ps8 = [pp.tile([P, MM_N], mybir.dt.float32,
                                   tag="ps", name="ps8") for _ in range(NH)]
                    ps16 = [pp.tile([P, MM_N], mybir.dt.float32,
                                    tag="ps", name="ps16") for _ in range(NH)]
                    for h in range(NH):
                        nc.tensor.matmul(
                            ps8[h][:], x8_t[:],
                            w8_t[:, :, h * MM_N:(h + 1) * MM_N],
                            start=True, stop=True,
                            perf_mode=mybir.MatmulPerfMode.DoubleRow)
                    for kk in range(KK16):
                        for h in range(NH):
                            nc.tensor.matmul(
                                ps16[h][:],
                                x_t[:, kk * P:(kk + 1) * P],
                                w_ts[kk][:, h * MM_N:(h + 1) * MM_N],
                                start=(kk == 0), stop=(kk == KK16 - 1))
                    for h in range(NH):
                        # o = ps8 * DESCALE + ps16.  HW allows only one
                        # PSUM operand per instruction, so stage through a
                        # small SBUF scratch tile.
                        s8 = op.tile([P, MM_N], mybir.dt.float32,
                                     tag="s8", bufs=4)
                        nc.vector.tensor_scalar_mul(
                            out=s8[:], in0=ps8[h][:], scalar1=DESCALE)
                        nc.vector.tensor_add(
                            out=o_t[:, h * MM_N:(h + 1) * MM_N],
                            in0=s8[:], in1=ps16[h][:])
                    nc.gpsimd.dma_start(
                        out=out[m0:m0 + P, :], in_=o_t[:])
                row0 += T * P
    nc.compile()
    return nc


_NC_CACHE = {}


def get_nc(t_list, K, NC_N, **kw):
    key = (tuple(t_list), K, NC_N, tuple(sorted(kw.items())))
    if key not in _NC_CACHE:
        _NC_CACHE[key] = build_nc(t_list, K, NC_N, **kw)
    return _NC_CACHE[key]


def pack_x(x_padded, order, t_e, K):
    """Swizzled x, fp16 part: row (chunk*P + p) = partition p's [kk16, m]
    line; fp8 part: partition p's [pair, m] line, pre-scaled by SX."""
    KK = K // P
    KK16 = KK - N8
    p16, p8 = [], []
    for e in order:
        T = t_e[e]
        xe = x_padded[e, :T * P, :]
        a = xe[:, N8 * P:].reshape(T, P, KK16, P).transpose(0, 3, 2, 1)
        p16.append(np.ascontiguousarray(a).reshape(T * P, KK16 * P))
        b = xe[:, :N8 * P].reshape(T, P, N8, P).transpose(0, 3, 2, 1)
        p8.append(np.ascontiguousarray(b).reshape(T * P, N8, P))
    return (to_fp16(np.concatenate(p16, axis=0)),
            to_fp8(np.concatenate(p8, axis=0), SX))


def pack_w(stacked_weights, order, c, NC_N, K):
    """Per-core swizzled weights.  fp16 part: row ((seg*KK16+kk)*P + p) =
    partition p's n-line; fp8 part: per segment, partition p's [pair, n]
    line, pre-scaled by SW."""
    KK = K // P
    KK16 = KK - N8
    p16, p8 = [], []
    for e in order:
        blk = stacked_weights[e, c * NC_N:(c + 1) * NC_N, :]
        a = blk[:, N8 * P:].reshape(NC_N, KK16, P).transpose(1, 2, 0)
        p16.append(np.ascontiguousarray(a).reshape(KK16 * P, NC_N))
        b = blk[:, :N8 * P].reshape(NC_N, N8, P).transpose(2, 1, 0)
        p8.append(np.ascontiguousarray(b))
    return (to_fp16(np.concatenate(p16, axis=0)),
            to_fp8(np.concatenate(p8, axis=0), SW))


def kernel(x_padded, stacked_weights, m_sizes):
    global LAST_RESULT
    x_padded = np.ascontiguousarray(np.asarray(x_padded, dtype=np.float32))
    stacked_weights = np.ascontiguousarray(
        np.asarray(stacked_weights, dtype=np.float32))
    E, MAX_M, K = x_padded.shape
    N = stacked_weights.shape[1]
    NC_N = N // N_CORES
    m = np.asarray(m_sizes).astype(np.int64)
    t_e = [min(int(math.ceil(mm / P)), (MAX_M + P - 1) // P) if mm > 0 else 0
           for mm in m]

    out_full = np.zeros((E, MAX_M, N), dtype=np.float32)
    order = [e for e in range(E) if t_e[e] > 0]
    if not order:
        return out_full
    # descending size: big early segments build weight-prefetch credit
    # that carries the small trailing segments without PE stalls
    order.sort(key=lambda e: -t_e[e])
    t_list = [t_e[e] for e in order]

    _install_profile_shim()
    nc = get_nc(t_list, K, NC_N)

    x16, x8 = pack_x(x_padded, order, t_e, K)
    in_maps = []
    for c in range(N_CORES):
        w16, w8 = pack_w(stacked_weights, order, c, NC_N, K)
        in_maps.append({"xsw16": x16, "xsw8": x8, "wsw16": w16, "wsw8": w8})

    res = run_bass_kernel_spmd(nc, in_maps, list(range(N_CORES)))
    LAST_RESULT = res

    for c in range(N_CORES):
        o = res.results[c]["out"]
        row = 0
        for i, e in enumerate(order):
            rows = int(min(m[e], MAX_M))
            out_full[e, :rows, c * NC_N:(c + 1) * NC_N] = o[row:row + rows]
            row += t_list[i] * P
    return out_full
